# revision 23
# baseline (speedup 1.0000x reference)
"""Trainium2 Bass kernel for nn_Attention_40570261078258.

Computes, for x:(8,128,64,64), Wq/Wk/Wv:(128,128), bq/bk/bv:(128,):
    xf = x.reshape(N, C, L);  L = 4096
    q/k/v = W @ xf + b                  -> (N, L, C) logical
    scores = q @ k^T / sqrt(C)          -> (N, L, L)
    attn = softmax(scores, axis=0)      # over the BATCH axis (torch legacy dim=0)
    out = attn @ v                      -> (N, L, C)
    return x + out.reshape(N, C, H, W)  # reinterpreting (L,C) memory as (C,H,W)

Sharding: softmax couples all batch elements at each (l, m), so we shard the
query dim L across the 8 cores; each core handles 512 query positions for all
batch elements (softmax fully local). Each core redundantly projects k/v for
all of L.

Schedule (fused, m-chunk-major): the host passes x pre-rotated along l so each
core's own 512 query columns are the FIRST four 128-column m-chunks of its
sweep. Pass A walks the 32 m-chunks doing: x DMA -> k/v (+q for the first 4
chunks) projections -> scores/exp/softmax/AV for l-slice 0, AV accumulating in
PSUM across the sweep. Pass B repeats the attention for l-slice 1 (PSUM can
only hold one slice's accumulators next to the double-buffered score tiles).
Engine budget per chunk (cost-model rates): PE 3.0us matmul in pass A; exp is
ACT-only (0.83ns/col); PSUM evictions are DVE/ACT-only (GPSIMD cannot touch
PSUM); z-sum/recip/normalize-mul on DVE (bf16 2x mode) with the zr-add and
half the normalize-mul pushed to the otherwise-idle GPSIMD; x DMAs alternate
between the SP and GPSIMD DGE queues because a queue's sequencer is held for
the whole transfer.
"""

import math

import numpy as np

import concourse.bacc as bacc
import concourse.bass as bass
import concourse.mybir as mybir
import concourse.tile as tile
from concourse.bass_utils import run_bass_kernel_spmd

N, C, H, W = 8, 128, 64, 64
L = H * W            # 4096 pixels
NCORES = 8
LSH = L // NCORES    # 512 query positions per core
NSL = 2              # l-slices per core
SW = LSH // NSL      # 256 l per slice
NMT = L // 128       # 32 m-chunks of 128
NQP = LSH // 128     # 4 m-chunks holding this core's own query columns

FP = mybir.dt.float32
FR = mybir.dt.float32r
BF = mybir.dt.bfloat16
AF = mybir.ActivationFunctionType
ALU = mybir.AluOpType

# Set by test harness to capture a profile.
TRACE = False
LAST_RESULTS = None
DEBUG_DUMP = False

XIN_BUFS = 4
DMA_SKEW = 3      # x-DMA runs this many chunks ahead of the projections
SX_SKEW = 1       # scores/exp lag behind projections by this many chunks
AV_SKEW = 1       # softmax/AV lag behind scores/exp


def build():
    nc = bacc.Bacc(
        "TRN2",
        target_bir_lowering=False,
        debug=False,
        enable_asserts=True,
        num_devices=NCORES,
    )

    # x arrives pre-rotated per core (own l-slice first); float32r bits so the
    # projection matmuls run at full PE rate without a bf16 pre-cast.
    xk = nc.dram_tensor("xk", [N, C, L], FR, kind="ExternalInput").ap()
    # Weights arrive pre-transposed from the host: w*t[c, o] = W[o, c].
    wq = nc.dram_tensor("wqt", [C, C], FR, kind="ExternalInput").ap()
    wk = nc.dram_tensor("wkt", [C, C], FR, kind="ExternalInput").ap()
    wv = nc.dram_tensor("wvt", [C, C], FR, kind="ExternalInput").ap()
    bq = nc.dram_tensor("bq", [C, 1], FP, kind="ExternalInput").ap()
    bk = nc.dram_tensor("bk", [C, 1], FP, kind="ExternalInput").ap()
    bv = nc.dram_tensor("bv", [1, C], FP, kind="ExternalInput").ap()
    # Attention output in (c, l)-major layout per l-slice; host reinterleaves
    # and adds the residual (pure glue, 0.4% of the FLOPs).
    out = nc.dram_tensor("out", [NSL, N, C, SW], FP, kind="ExternalOutput").ap()
    dbg = None
    if DEBUG_DUMP:
        dbg = {
            "dbg_q": nc.dram_tensor("dbg_q", [C, N * LSH], FP, kind="ExternalOutput").ap(),
            "dbg_k": nc.dram_tensor("dbg_k", [C, NMT * N * 128], FP, kind="ExternalOutput").ap(),
            "dbg_v": nc.dram_tensor("dbg_v", [128, NMT * N * 128], FP, kind="ExternalOutput").ap(),
        }

    with tile.TileContext(nc) as tc:
        _emit(nc, tc, xk, wq, wk, wv, bq, bk, bv, out, dbg)

    nc.compile()
    return nc


def _emit(nc, tc, xk, wq, wk, wv, bq, bk, bv, out, dbg=None):
    from contextlib import ExitStack

    inv_sqrt_c = 1.0 / math.sqrt(C)

    with ExitStack() as ctx:
        cpool = ctx.enter_context(tc.tile_pool(name="const", bufs=1))
        resid = ctx.enter_context(tc.tile_pool(name="resident", bufs=1))

        # --- constants -----------------------------------------------------
        bq_t = cpool.tile([C, 1], FP, tag="bq")
        nc.sync.dma_start(bq_t[:], bq)
        bk_t = cpool.tile([C, 1], FP, tag="bk")
        nc.sync.dma_start(bk_t[:], bk)
        bv_f = cpool.tile([1, C], FP, tag="bvf")
        nc.sync.dma_start(bv_f[:], bv)
        ones_row = cpool.tile([1, C], FP, tag="ones")
        nc.vector.memset(ones_row[:], 1.0)
        bv_rep = cpool.tile([128, C], FP, tag="bvrep")

        wT = {}
        with tc.tile_pool(name="wpsum", bufs=1, space="PSUM") as wpsum_pool:
            for name, wap in (("q", wq), ("k", wk), ("v", wv)):
                wt = cpool.tile([C, C], FR, tag=f"w{name}T")
                nc.sync.dma_start(wt[:], wap)
                wT[name] = wt
            pb = wpsum_pool.tile([128, C], FP, tag="wps")
            nc.tensor.matmul(pb[:], ones_row[:], bv_f[:], start=True, stop=True)
            nc.vector.tensor_copy(bv_rep[:], pb[:])
        # WvT padded to 256 columns of zeros so the float32r vT matmuls hit
        # the >=256 free-dim full-rate path (junk half never read).
        wvpad = cpool.tile([C, 2 * C], FR, tag="wvpad")
        zpad = cpool.tile([C, 2 * C], FP, tag="zpad")
        nc.vector.memset(zpad[:], 0.0)
        nc.vector.tensor_copy(wvpad[:], zpad[:])
        nc.vector.tensor_copy(wvpad[:, 0:C], wT["v"][:])
        wqT_r = wT["q"][:]
        wkT_r = wT["k"][:]
        wvpad_r = wvpad[:]

        # --- resident activations -----------------------------------------
        # k_big: (c, p*N*128) keys for every (chunk, n);  vT_big: (m_local,
        # p*N*128) values transposed;  q_sb: (c, n*LSH) this core's queries.
        k_big = resid.tile([C, NMT * N * 128], BF, tag="kbig", name="k_big")
        vT_big = resid.tile([128, NMT * N * 128], BF, tag="vbig", name="vT_big")
        q_sb = resid.tile([C, N * LSH], BF, tag="qsb", name="q_sb")

        xin_pool = ctx.enter_context(tc.tile_pool(name="xin", bufs=1))
        soft_pool = ctx.enter_context(tc.tile_pool(name="soft", bufs=1))
        ost_pool = ctx.enter_context(tc.tile_pool(name="ost", bufs=1))

        xts = {}

        def emit_dma(p):
            xt = xin_pool.tile([C, N * 128], FR, tag="x", bufs=XIN_BUFS)
            # Both APs must lead with the partition dim for correct DMA
            # descriptor generation. Alternate DGE queues: the issuing
            # sequencer is held for the whole transfer.
            eng = nc.sync if p % 2 == 0 else nc.gpsimd
            eng.dma_start(
                xt[:].rearrange("c (n l) -> c n l", n=N),
                xk[:, :, p * 128 : (p + 1) * 128].rearrange("n c l -> c n l"),
            )
            xts[p] = xt

        def emit_proj(p, tr_pool):
            """k/v (and for p<NQP q) projections for m-chunk p."""
            xt = xts.pop(p)
            # k: out[c_out, (n,l)] = sum_c' WkT[c',c_out] x[c', (n,l)];
            # FD=256 per matmul (n-pairs) keeps float32r at full rate.
            tk = tr_pool.tile([128, 1024], FP, tag="tr", bufs=2)
            for np_ in range(4):
                sl = slice(np_ * 256, np_ * 256 + 256)
                nc.tensor.matmul(
                    tk[:, sl], wkT_r, xt[:, sl], start=True, stop=True
                )
            # ACT absorbs the k eviction (bias bk is per-partition here).
            nc.scalar.activation(
                k_big[:, p * 1024 : (p + 1) * 1024], tk[:], AF.Identity,
                bias=bk_t[:],
            )
            # v: out[m_local, c] per n; stationary x-chunk, moving padded WvT.
            for vt_i in range(2):
                tv = tr_pool.tile([128, 1024], FP, tag="tr", bufs=2)
                for sub in range(4):
                    n = 4 * vt_i + sub
                    nc.tensor.matmul(
                        tv[:, sub * 256 : (sub + 1) * 256],
                        xt[:, n * 128 : (n + 1) * 128],
                        wvpad_r,
                        start=(sub % 2 == 0),
                        stop=(sub % 2 == 1),
                    )
                # GPSIMD cannot read PSUM; the junk-skipping strided eviction
                # with the bv broadcast-add must run on DVE.
                nc.vector.scalar_tensor_tensor(
                    vT_big[
                        :, (p * N + 4 * vt_i) * 128 : (p * N + 4 * vt_i + 4) * 128
                    ].rearrange("p (s c) -> p s c", s=4),
                    tv[:].rearrange("p (s c2) -> p s c2", s=4)[:, :, 0:128],
                    1.0,
                    bv_rep[:].unsqueeze(1).broadcast_to((128, 4, C)),
                    ALU.mult,
                    ALU.add,
                )
            # q for this core's own chunks (positions 0..NQP-1 post-rotation).
            if p < NQP:
                tq = tr_pool.tile([128, 1024], FP, tag="tr", bufs=2)
                for np_ in range(4):
                    sl = slice(np_ * 256, np_ * 256 + 256)
                    nc.tensor.matmul(
                        tq[:, sl], wqT_r, xt[:, sl], start=True, stop=True
                    )
                # dst: q_sb[(c, n, l)] for all n, l-chunk p (128 wide).
                nc.scalar.activation(
                    q_sb[:]
                    .rearrange("c (n l) -> c n l", n=N)[
                        :, :, p * 128 : (p + 1) * 128
                    ],
                    tq[:].rearrange("c (n l) -> c n l", n=N),
                    AF.Identity,
                    bias=bq_t[:],
                )

        def emit_scores_exp(s, p, sc_pool, pend):
            """scores + exp for l-slice s at m-chunk p -> E tile (m, n*SW)."""
            e = soft_pool.tile([128, N * SW], BF, tag="E", bufs=3)
            for t in range(2):
                ps = sc_pool.tile([128, 1024], FP, tag="tr", bufs=2)
                for i in range(4):
                    n = 4 * t + i
                    nc.tensor.matmul(
                        ps[:, i * SW : (i + 1) * SW],
                        k_big[:, (p * N + n) * 128 : (p * N + n + 1) * 128],
                        q_sb[:, n * LSH + s * SW : n * LSH + (s + 1) * SW],
                        start=True,
                        stop=True,
                    )
                nc.scalar.activation(
                    e[:, t * 1024 : (t + 1) * 1024],
                    ps[:],
                    AF.Exp,
                    scale=inv_sqrt_c,
                )
            pend[(s, p)] = e

        def emit_soft_av(s, p, avp, pend, pool_zr):
            """z-sum over n, reciprocal, normalize, AV accumulate.

            pool_zr moves the final z add to GPSIMD (pass A relief); half the
            normalize mul always runs there.
            """
            e = pend.pop((s, p))
            s1 = soft_pool.tile([128, 4 * SW], BF, tag="zt1", bufs=2)
            nc.vector.tensor_add(s1[:], e[:, 0 : 4 * SW], e[:, 4 * SW : 8 * SW])
            s2 = soft_pool.tile([128, 2 * SW], BF, tag="zt2", bufs=2)
            nc.vector.tensor_add(s2[:], s1[:, 0 : 2 * SW], s1[:, 2 * SW : 4 * SW])
            zr = soft_pool.tile([128, SW], BF, tag="zr", bufs=2)
            zr_eng = nc.gpsimd if pool_zr else nc.vector
            zr_eng.tensor_add(zr[:], s2[:, 0:SW], s2[:, SW : 2 * SW])
            r = soft_pool.tile([128, SW], BF, tag="r", bufs=2)
            with nc.allow_low_precision(
                "softmax denom is a sum of 8 O(1..500) exps; bf16 ok"
            ):
                nc.vector.reciprocal(r[:], zr[:])
            a = soft_pool.tile([128, N * SW], BF, tag="A", bufs=2)
            half = N // 2
            nc.vector.tensor_mul(
                a[:, : half * SW].rearrange("p (g l) -> p g l", g=half),
                e[:, : half * SW].rearrange("p (g l) -> p g l", g=half),
                r[:].unsqueeze(1).broadcast_to((128, half, SW)),
            )
            nc.gpsimd.tensor_mul(
                a[:, half * SW :].rearrange("p (g l) -> p g l", g=half),
                e[:, half * SW :].rearrange("p (g l) -> p g l", g=half),
                r[:].unsqueeze(1).broadcast_to((128, half, SW)),
            )
            for n in range(N):
                # start clears the whole bank's has_written bits: only the
                # first matmul of the accumulation group per bank may set it.
                nc.tensor.matmul(
                    avp[:, n * SW : (n + 1) * SW],
                    vT_big[:, (p * N + n) * 128 : (p * N + n + 1) * 128],
                    a[:, n * SW : (n + 1) * SW],
                    start=(p == 0 and n % 2 == 0),
                    stop=(p == NMT - 1 and n % 2 == 1),
                )

        def emit_epilogue(s, avp):
            ob = ost_pool.tile([128, N * SW], FP, tag="ob", bufs=2)
            nc.vector.tensor_copy(ob[:], avp)
            nc.sync.dma_start(
                out[s].rearrange("n c l -> c n l"),
                ob[:].rearrange("c (n l) -> c n l", n=N),
            )

        # === pass A: projections + l-slice 0 ================================
        with (
            tc.tile_pool(name="trA", bufs=1, space="PSUM") as trA,
            tc.tile_pool(name="avA", bufs=1, space="PSUM") as avA,
        ):
            avp = avA.tile([128, N * SW], FP, tag="av", name="avpA")
            pend = {}
            for t in range(NMT + DMA_SKEW + SX_SKEW + AV_SKEW):
                if t < NMT:
                    emit_dma(t)
                p1 = t - DMA_SKEW
                if 0 <= p1 < NMT:
                    emit_proj(p1, trA)
                p2 = p1 - SX_SKEW
                if 0 <= p2 < NMT:
                    emit_scores_exp(0, p2, trA, pend)
                p3 = p2 - AV_SKEW
                if 0 <= p3 < NMT:
                    emit_soft_av(0, p3, avp[:], pend, pool_zr=True)
            emit_epilogue(0, avp[:])

        if dbg is not None:
            # bf16 -> fp32 staging then DMA out for host-side verification.
            with tc.tile_pool(name="dbgp", bufs=1) as dp:
                for blk in range(2):
                    sl = slice(blk * 2048, (blk + 1) * 2048)
                    dq = dp.tile([C, 2048], FP, tag="dt", bufs=2)
                    nc.vector.tensor_copy(dq[:], q_sb[:, sl])
                    nc.sync.dma_start(dbg["dbg_q"][:, sl], dq[:])
                for blk in range(16):
                    sl = slice(blk * 2048, (blk + 1) * 2048)
                    dk = dp.tile([C, 2048], FP, tag="dt", bufs=2)
                    nc.vector.tensor_copy(dk[:], k_big[:, sl])
                    nc.sync.dma_start(dbg["dbg_k"][:, sl], dk[:])
                    dv = dp.tile([128, 2048], FP, tag="dt", bufs=2)
                    nc.vector.tensor_copy(dv[:], vT_big[:, sl])
                    nc.sync.dma_start(dbg["dbg_v"][:, sl], dv[:])

        # === pass B: l-slice 1 ==============================================
        with (
            tc.tile_pool(name="trB", bufs=1, space="PSUM") as trB,
            tc.tile_pool(name="avB", bufs=1, space="PSUM") as avB,
        ):
            avp = avB.tile([128, N * SW], FP, tag="av", name="avpB")
            pend = {}
            for t in range(NMT + 2):
                if t < NMT:
                    emit_scores_exp(1, t, trB, pend)
                if 0 <= t - 2 < NMT:
                    emit_soft_av(1, t - 2, avp[:], pend, pool_zr=False)
            emit_epilogue(1, avp[:])


_NC = None


def _get_nc():
    global _NC
    if _NC is None:
        _NC = build()
    return _NC


def kernel(x, Wq, bq, Wk, bk, Wv, bv):
    global LAST_RESULTS
    x = np.ascontiguousarray(np.asarray(x, dtype=np.float32))
    WqT = np.ascontiguousarray(np.asarray(Wq, dtype=np.float32).T)
    WkT = np.ascontiguousarray(np.asarray(Wk, dtype=np.float32).T)
    WvT = np.ascontiguousarray(np.asarray(Wv, dtype=np.float32).T)
    bq = np.asarray(bq, dtype=np.float32).reshape(C, 1)
    bk = np.asarray(bk, dtype=np.float32).reshape(C, 1)
    bv = np.asarray(bv, dtype=np.float32).reshape(1, C)

    xf = x.reshape(N, C, L)
    xflat = x.reshape(N, C * H * W)

    in_maps = []
    for d in range(NCORES):
        lo = d * LSH
        # Rotate l so this core's own query columns are chunks 0..NQP-1.
        xrot = np.ascontiguousarray(
            np.concatenate([xf[:, :, lo:], xf[:, :, :lo]], axis=2)
        )
        in_maps.append(
            {
                "xk": xrot,
                "wqt": WqT,
                "wkt": WkT,
                "wvt": WvT,
                "bq": bq,
                "bk": bk,
                "bv": bv,
            }
        )

    nc = _get_nc()
    res = run_bass_kernel_spmd(
        nc, in_maps, core_ids=list(range(NCORES)), trace=TRACE
    )
    LAST_RESULTS = res
    # Device returns (NSL, N, C, SW) per core; reinterleave to the reference's
    # flat (l, c) order and add the residual here.
    att = np.concatenate(
        [
            res.results[d]["out"].transpose(1, 0, 3, 2).reshape(N, LSH * C)
            for d in range(NCORES)
        ],
        axis=1,
    )
    return (xflat + att).reshape(N, C, H, W)


# revision 36
# speedup vs baseline: 1.1297x; 1.1297x over previous
"""Trainium2 Bass kernel for nn_Attention_40570261078258.

Computes, for x:(8,128,64,64), Wq/Wk/Wv:(128,128), bq/bk/bv:(128,):
    xf = x.reshape(N, C, L);  L = 4096
    q/k/v = W @ xf + b                  -> (N, L, C) logical
    scores = q @ k^T / sqrt(C)          -> (N, L, L)
    attn = softmax(scores, axis=0)      # over the BATCH axis (torch legacy dim=0)
    out = attn @ v                      -> (N, L, C)
    return x + out.reshape(N, C, H, W)  # reinterpreting (L,C) memory as (C,H,W)

Sharding: softmax couples all batch elements at each (l, m), so we shard the
query dim L across the 8 cores; each core handles 512 query positions for all
batch elements (softmax fully local). Each core redundantly projects k/v for
all of L.

Schedule (fused, m-chunk-major): the host passes x pre-rotated along l so each
core's own 512 query columns are the FIRST four 128-column m-chunks of its
sweep. Pass A walks the 32 m-chunks doing: x DMA -> k/v (+q for the first 4
chunks) projections -> scores/exp/softmax/AV for l-slice 0, AV accumulating in
PSUM across the sweep. Pass B repeats the attention for l-slice 1 (PSUM can
only hold one slice's accumulators next to the double-buffered score tiles).
Engine budget per chunk (cost-model rates): PE 3.0us matmul in pass A; exp is
ACT-only (0.83ns/col); PSUM evictions are DVE/ACT-only (GPSIMD cannot touch
PSUM); z-sum/recip/normalize-mul on DVE (bf16 2x mode) with the zr-add and
half the normalize-mul pushed to the otherwise-idle GPSIMD; x DMAs alternate
between the SP and GPSIMD DGE queues because a queue's sequencer is held for
the whole transfer.
"""

import math

import numpy as np

import concourse.bacc as bacc
import concourse.bass as bass
import concourse.mybir as mybir
import concourse.tile as tile
from concourse.bass_utils import run_bass_kernel_spmd

N, C, H, W = 8, 128, 64, 64
L = H * W            # 4096 pixels
NCORES = 8
LSH = L // NCORES    # 512 query positions per core
NSL = 2              # l-slices per core
SW = LSH // NSL      # 256 l per slice
NMT = L // 128       # 32 m-chunks of 128
NQP = LSH // 128     # 4 m-chunks holding this core's own query columns

FP = mybir.dt.float32
FR = mybir.dt.float32r
BF = mybir.dt.bfloat16
AF = mybir.ActivationFunctionType
ALU = mybir.AluOpType

# Set by test harness to capture a profile.
TRACE = False
LAST_RESULTS = None
DEBUG_DUMP = False

XIN_BUFS = 3      # x staging tiles (DMA_CHUNKS chunks each)
DMA_CHUNKS = 2    # m-chunks per x DMA (bigger transfers, fewer queue holds)


def build():
    nc = bacc.Bacc(
        "TRN2",
        target_bir_lowering=False,
        debug=False,
        enable_asserts=True,
        num_devices=NCORES,
    )

    # x arrives pre-rotated per core (own l-slice first); float32r bits so the
    # projection matmuls run at full PE rate without a bf16 pre-cast.
    xk = nc.dram_tensor("xk", [N, C, L], FR, kind="ExternalInput").ap()
    # Weights arrive pre-transposed from the host: w*t[c, o] = W[o, c].
    wq = nc.dram_tensor("wqt", [C, C], FR, kind="ExternalInput").ap()
    wk = nc.dram_tensor("wkt", [C, C], FR, kind="ExternalInput").ap()
    wv = nc.dram_tensor("wvt", [C, C], FR, kind="ExternalInput").ap()
    bq = nc.dram_tensor("bq", [C, 1], FP, kind="ExternalInput").ap()
    bk = nc.dram_tensor("bk", [C, 1], FP, kind="ExternalInput").ap()
    bv = nc.dram_tensor("bv", [1, C], FP, kind="ExternalInput").ap()
    # Attention output in (c, l)-major layout per l-slice; host reinterleaves
    # and adds the residual (pure glue, 0.4% of the FLOPs).
    out = nc.dram_tensor("out", [NSL, N, C, SW], FP, kind="ExternalOutput").ap()
    dbg = None
    if DEBUG_DUMP:
        dbg = {
            "dbg_q": nc.dram_tensor("dbg_q", [C, N * LSH], FP, kind="ExternalOutput").ap(),
            "dbg_k": nc.dram_tensor("dbg_k", [C, NMT * N * 128], FP, kind="ExternalOutput").ap(),
            "dbg_v": nc.dram_tensor("dbg_v", [128, NMT * N * 128], FP, kind="ExternalOutput").ap(),
        }

    with tile.TileContext(nc) as tc:
        _emit(nc, tc, xk, wq, wk, wv, bq, bk, bv, out, dbg)

    nc.compile()
    return nc


def _emit(nc, tc, xk, wq, wk, wv, bq, bk, bv, out, dbg=None):
    from contextlib import ExitStack

    inv_sqrt_c = 1.0 / math.sqrt(C)

    with ExitStack() as ctx:
        cpool = ctx.enter_context(tc.tile_pool(name="const", bufs=1))
        resid = ctx.enter_context(tc.tile_pool(name="resident", bufs=1))

        # --- constants -----------------------------------------------------
        bq_t = cpool.tile([C, 1], FP, tag="bq")
        nc.sync.dma_start(bq_t[:], bq)
        bk_t = cpool.tile([C, 1], FP, tag="bk")
        nc.sync.dma_start(bk_t[:], bk)
        bv_f = cpool.tile([1, C], FP, tag="bvf")
        nc.sync.dma_start(bv_f[:], bv)
        ones_row = cpool.tile([1, C], FP, tag="ones")
        nc.vector.memset(ones_row[:], 1.0)
        bv_rep = cpool.tile([128, C], FP, tag="bvrep")

        wT = {}
        with tc.tile_pool(name="wpsum", bufs=1, space="PSUM") as wpsum_pool:
            for name, wap in (("q", wq), ("k", wk), ("v", wv)):
                wt = cpool.tile([C, C], FR, tag=f"w{name}T")
                nc.sync.dma_start(wt[:], wap)
                wT[name] = wt
            pb = wpsum_pool.tile([128, C], FP, tag="wps")
            nc.tensor.matmul(pb[:], ones_row[:], bv_f[:], start=True, stop=True)
            nc.vector.tensor_copy(bv_rep[:], pb[:])
        # WvT padded to 256 columns of zeros so the float32r vT matmuls hit
        # the >=256 free-dim full-rate path (junk half never read).
        wvpad = cpool.tile([C, 2 * C], FR, tag="wvpad")
        zpad = cpool.tile([C, 2 * C], FP, tag="zpad")
        nc.vector.memset(zpad[:], 0.0)
        nc.vector.tensor_copy(wvpad[:], zpad[:])
        nc.vector.tensor_copy(wvpad[:, 0:C], wT["v"][:])
        wqT_r = wT["q"][:]
        wkT_r = wT["k"][:]
        wvpad_r = wvpad[:]

        # --- resident activations -----------------------------------------
        # k_big: (c, p*N*128) keys for every (chunk, n);  vT_big: (m_local,
        # p*N*128) values transposed;  q_sb: (c, n*LSH) this core's queries.
        k_big = resid.tile([C, NMT * N * 128], BF, tag="kbig", name="k_big")
        vT_big = resid.tile([128, NMT * N * 128], BF, tag="vbig", name="vT_big")
        q_sb = resid.tile([C, N * LSH], BF, tag="qsb", name="q_sb")

        xin_pool = ctx.enter_context(tc.tile_pool(name="xin", bufs=1))
        soft_pool = ctx.enter_context(tc.tile_pool(name="soft", bufs=1))
        ost_pool = ctx.enter_context(tc.tile_pool(name="ost", bufs=1))

        xts = {}
        BW = DMA_CHUNKS * 128  # x columns per block and per n

        def kvcol(p, n):
            """Column of chunk (p, n) in k_big/vT_big (block-major layout)."""
            return (p // DMA_CHUNKS) * N * BW + n * BW + (p % DMA_CHUNKS) * 128

        def emit_dma(blk):
            """One DMA per block: xt[c, n*BW + j*128 + l] = x[n, c, blk*BW+...]."""
            xt = xin_pool.tile([C, N * BW], FR, tag="x", bufs=XIN_BUFS)
            # Both APs must lead with the partition dim for correct DMA
            # descriptor generation. Alternate DGE queues: the issuing
            # sequencer is held for the whole transfer.
            eng = nc.sync if blk % 2 == 0 else nc.gpsimd
            eng.dma_start(
                xt[:].rearrange("c (n w) -> c n w", n=N),
                xk[:, :, blk * BW : (blk + 1) * BW].rearrange("n c w -> c n w"),
            )
            xts[blk] = xt

        def emit_proj(blk, tr_pool):
            """k/v (and q for the first blocks) projections for one block."""
            xt = xts.pop(blk)
            # k: out[c_out, (n,w)] = sum_c' WkT[c',c_out] x[c', (n,w)]; each
            # matmul covers one n's BW=256 columns (float32r full rate).
            for t in range(2):
                tk = tr_pool.tile([128, 1024], FP, tag="tr", bufs=4)
                for i in range(4):
                    n = 4 * t + i
                    nc.tensor.matmul(
                        tk[:, i * BW : (i + 1) * BW],
                        wkT_r,
                        xt[:, n * BW : (n + 1) * BW],
                        start=True,
                        stop=True,
                    )
                # ACT absorbs the k eviction (bias bk is per-partition here).
                nc.scalar.activation(
                    k_big[
                        :, blk * N * BW + t * 4 * BW : blk * N * BW + (t + 1) * 4 * BW
                    ],
                    tk[:],
                    AF.Identity,
                    bias=bk_t[:],
                )
            # v: out[m_local, c] per (n, chunk); stationary 128-col x slice,
            # moving padded WvT (256 wide, junk half skipped at eviction).
            for vt_i in range(4):
                tv = tr_pool.tile([128, 1024], FP, tag="tr", bufs=4)
                for sub in range(4):
                    nj = 4 * vt_i + sub  # flat (n, j) index
                    nc.tensor.matmul(
                        tv[:, sub * 256 : (sub + 1) * 256],
                        xt[:, nj * 128 : (nj + 1) * 128],
                        wvpad_r,
                        start=(sub % 2 == 0),
                        stop=(sub % 2 == 1),
                    )
                # GPSIMD cannot read PSUM; the junk-skipping strided eviction
                # with the bv broadcast-add must run on DVE.
                nc.vector.scalar_tensor_tensor(
                    vT_big[
                        :,
                        blk * N * BW + vt_i * 512 : blk * N * BW + (vt_i + 1) * 512,
                    ].rearrange("p (s c) -> p s c", s=4),
                    tv[:].rearrange("p (s c2) -> p s c2", s=4)[:, :, 0:128],
                    1.0,
                    bv_rep[:].unsqueeze(1).broadcast_to((128, 4, C)),
                    ALU.mult,
                    ALU.add,
                )
            # q for this core's own columns (first NQP chunks post-rotation).
            if blk * DMA_CHUNKS < NQP:
                for t in range(2):
                    tq = tr_pool.tile([128, 1024], FP, tag="tr", bufs=4)
                    for i in range(4):
                        n = 4 * t + i
                        nc.tensor.matmul(
                            tq[:, i * BW : (i + 1) * BW],
                            wqT_r,
                            xt[:, n * BW : (n + 1) * BW],
                            start=True,
                            stop=True,
                        )
                    # dst: q_sb[(c, n, w)] for n in [4t, 4t+4), w-block blk.
                    nc.scalar.activation(
                        q_sb[:]
                        .rearrange("c (n l) -> c n l", n=N)[
                            :, 4 * t : 4 * t + 4, blk * BW : (blk + 1) * BW
                        ],
                        tq[:].rearrange("c (n w) -> c n w", n=4),
                        AF.Identity,
                        bias=bq_t[:],
                    )

        def emit_scores_exp(s, p, sc_pool, pend):
            """scores + exp for l-slice s at m-chunk p -> E tile (m, n*SW)."""
            e = soft_pool.tile([128, N * SW], BF, tag="E", bufs=3)
            for t in range(2):
                ps = sc_pool.tile([128, 1024], FP, tag="tr", bufs=2)
                for i in range(4):
                    n = 4 * t + i
                    nc.tensor.matmul(
                        ps[:, i * SW : (i + 1) * SW],
                        k_big[:, kvcol(p, n) : kvcol(p, n) + 128],
                        q_sb[:, n * LSH + s * SW : n * LSH + (s + 1) * SW],
                        start=True,
                        stop=True,
                    )
                nc.scalar.activation(
                    e[:, t * 1024 : (t + 1) * 1024],
                    ps[:],
                    AF.Exp,
                    scale=inv_sqrt_c,
                )
            pend[(s, p)] = e

        def emit_soft_av(s, p, avp, pend):
            """z-sum over n, reciprocal, normalize, AV accumulate.

            The normalize mul splits 4 n-groups to GPSIMD (slow: emitted
            first) and 4 to DVE; each half feeds its own 4 AV matmuls so PE
            doesn't wait on the slower engine.
            """
            e = pend.pop((s, p))
            s1 = soft_pool.tile([128, 4 * SW], BF, tag="zt1", bufs=2)
            nc.vector.tensor_add(s1[:], e[:, 0 : 4 * SW], e[:, 4 * SW : 8 * SW])
            s2 = soft_pool.tile([128, 2 * SW], BF, tag="zt2", bufs=2)
            nc.vector.tensor_add(s2[:], s1[:, 0 : 2 * SW], s1[:, 2 * SW : 4 * SW])
            zr = soft_pool.tile([128, SW], BF, tag="zr", bufs=2)
            nc.vector.tensor_add(zr[:], s2[:, 0:SW], s2[:, SW : 2 * SW])
            r = soft_pool.tile([128, SW], BF, tag="r", bufs=2)
            with nc.allow_low_precision(
                "softmax denom is a sum of 8 O(1..500) exps; bf16 ok"
            ):
                nc.vector.reciprocal(r[:], zr[:])
            a = soft_pool.tile([128, N * SW], BF, tag="A", bufs=2)
            half = N // 2

            def mul_av(eng, lo):
                eng.tensor_mul(
                    a[:, lo * SW : (lo + half) * SW].rearrange(
                        "p (g l) -> p g l", g=half
                    ),
                    e[:, lo * SW : (lo + half) * SW].rearrange(
                        "p (g l) -> p g l", g=half
                    ),
                    r[:].unsqueeze(1).broadcast_to((128, half, SW)),
                )
                for n in range(lo, lo + half):
                    # start clears the whole bank's has_written bits: only
                    # the first matmul of the group per bank may set it.
                    nc.tensor.matmul(
                        avp[:, n * SW : (n + 1) * SW],
                        vT_big[:, kvcol(p, n) : kvcol(p, n) + 128],
                        a[:, n * SW : (n + 1) * SW],
                        start=(p == 0 and n % 2 == 0),
                        stop=(p == NMT - 1 and n % 2 == 1),
                    )

            mul_av(nc.gpsimd, half)  # slow engine first
            mul_av(nc.vector, 0)

        def emit_epilogue(s, avp):
            # Split the accumulator eviction across DVE and ACT.
            ob = ost_pool.tile([128, N * SW], FP, tag="ob", bufs=1)
            nc.vector.tensor_copy(ob[:, 0 : N * SW // 2], avp[:, 0 : N * SW // 2])
            nc.scalar.copy(ob[:, N * SW // 2 :], avp[:, N * SW // 2 :])
            nc.sync.dma_start(
                out[s].rearrange("n c l -> c n l"),
                ob[:].rearrange("c (n l) -> c n l", n=N),
            )

        # === phase P: all projections (DMA-bound) ===========================
        NBLK = NMT // DMA_CHUNKS
        with tc.tile_pool(name="trP", bufs=1, space="PSUM") as trP:
            for t in range(NBLK + 2):
                if t < NBLK:
                    emit_dma(t)
                if 0 <= t - 2 < NBLK:
                    emit_proj(t - 2, trP)

        # === passes S0/S1: attention per l-slice ============================
        def attention_pass(s):
            with (
                tc.tile_pool(name=f"sc{s}", bufs=1, space="PSUM") as scp,
                tc.tile_pool(name=f"av{s}", bufs=1, space="PSUM") as avP,
            ):
                avp = avP.tile([128, N * SW], FP, tag="av", name=f"avp{s}")
                pend = {}
                for t in range(NMT + 2):
                    if t < NMT:
                        emit_scores_exp(s, t, scp, pend)
                    if 0 <= t - 2 < NMT:
                        emit_soft_av(s, t - 2, avp[:], pend)
                emit_epilogue(s, avp[:])

        attention_pass(0)

        if dbg is not None:
            # bf16 -> fp32 staging then DMA out for host-side verification.
            with tc.tile_pool(name="dbgp", bufs=1) as dp:
                for blk in range(2):
                    sl = slice(blk * 2048, (blk + 1) * 2048)
                    dq = dp.tile([C, 2048], FP, tag="dt", bufs=2)
                    nc.vector.tensor_copy(dq[:], q_sb[:, sl])
                    nc.sync.dma_start(dbg["dbg_q"][:, sl], dq[:])
                for blk in range(16):
                    sl = slice(blk * 2048, (blk + 1) * 2048)
                    dk = dp.tile([C, 2048], FP, tag="dt", bufs=2)
                    nc.vector.tensor_copy(dk[:], k_big[:, sl])
                    nc.sync.dma_start(dbg["dbg_k"][:, sl], dk[:])
                    dv = dp.tile([128, 2048], FP, tag="dt", bufs=2)
                    nc.vector.tensor_copy(dv[:], vT_big[:, sl])
                    nc.sync.dma_start(dbg["dbg_v"][:, sl], dv[:])

        attention_pass(1)


_NC = None


def _get_nc():
    global _NC
    if _NC is None:
        _NC = build()
    return _NC


def kernel(x, Wq, bq, Wk, bk, Wv, bv):
    global LAST_RESULTS
    x = np.ascontiguousarray(np.asarray(x, dtype=np.float32))
    WqT = np.ascontiguousarray(np.asarray(Wq, dtype=np.float32).T)
    WkT = np.ascontiguousarray(np.asarray(Wk, dtype=np.float32).T)
    WvT = np.ascontiguousarray(np.asarray(Wv, dtype=np.float32).T)
    bq = np.asarray(bq, dtype=np.float32).reshape(C, 1)
    bk = np.asarray(bk, dtype=np.float32).reshape(C, 1)
    bv = np.asarray(bv, dtype=np.float32).reshape(1, C)

    xf = x.reshape(N, C, L)
    xflat = x.reshape(N, C * H * W)

    in_maps = []
    for d in range(NCORES):
        lo = d * LSH
        # Rotate l so this core's own query columns are chunks 0..NQP-1.
        xrot = np.ascontiguousarray(
            np.concatenate([xf[:, :, lo:], xf[:, :, :lo]], axis=2)
        )
        in_maps.append(
            {
                "xk": xrot,
                "wqt": WqT,
                "wkt": WkT,
                "wvt": WvT,
                "bq": bq,
                "bk": bk,
                "bv": bv,
            }
        )

    nc = _get_nc()
    res = run_bass_kernel_spmd(
        nc, in_maps, core_ids=list(range(NCORES)), trace=TRACE
    )
    LAST_RESULTS = res
    # Device returns (NSL, N, C, SW) per core; reinterleave to the reference's
    # flat (l, c) order and add the residual here.
    att = np.concatenate(
        [
            res.results[d]["out"].transpose(1, 0, 3, 2).reshape(N, LSH * C)
            for d in range(NCORES)
        ],
        axis=1,
    )
    return (xflat + att).reshape(N, C, H, W)
